# revision 1
# baseline (speedup 1.0000x reference)
"""Trainium2 Bass kernel for nn_CrossFusion (CBN + L2-norms + tiny-head cross-attention).

Self-contained: hardcodes shapes/sharding. Shards the S1 (query) axis across 8
NeuronCores; x2-side work (stats, k, v) is replicated per core. The attention
matrix is never materialized to HBM: scores are generated on the fly as
e = exp(q_s * k_t) with one ACT op per (s-chunk, head), the softmax denominator
comes free via the ACT accumulator, and the numerator is a fused
tensor_tensor_reduce against a broadcast v in bf16.

Layouts: all big tensors are column-form "(p c) d": row index = p*C + c, so a
[128, C*64] SBUF tile holds rows with partition p owning rows p*C..p*C+C-1.
The t-order of k/v/x2 and s-order of q/output use the same mapping, so the
softmax (a sum over all t) is order-invariant and outputs land contiguously.
"""
import numpy as np

S = 4096          # S1 == S2
D = 64
H = 2
NCORES = 8
SSH = S // NCORES  # 512 rows of s per core
SC = SSH // 128    # 4 s-chunks per core
TC = S // 128      # 32 t-chunks
EPS_BN = 1e-5

_CACHE = {}


def _build(split=True):
    import concourse.bass as bass
    import concourse.tile as tile
    import concourse.mybir as mybir

    f32 = mybir.dt.float32
    bf16 = mybir.dt.bfloat16
    AF = mybir.ActivationFunctionType
    ALU = mybir.AluOpType
    P = 128

    nc = bass.Bass("TRN2", target_bir_lowering=False, debug=False)

    x1s = nc.dram_tensor("x1s", [SSH, D], f32, kind="ExternalInput")
    x1f = nc.dram_tensor("x1f", [S, D], f32, kind="ExternalInput")
    x2 = nc.dram_tensor("x2", [S, D], f32, kind="ExternalInput")
    Wq = nc.dram_tensor("Wq", [D, H], f32, kind="ExternalInput")
    Wk = nc.dram_tensor("Wk", [D, H], f32, kind="ExternalInput")
    Wv = nc.dram_tensor("Wv", [D, H], f32, kind="ExternalInput")
    Wo = nc.dram_tensor("Wo", [H, 2], f32, kind="ExternalInput")
    bo = nc.dram_tensor("bo", [1, 2], f32, kind="ExternalInput")
    Wg1 = nc.dram_tensor("Wg1", [D, D], f32, kind="ExternalInput")
    Wg2 = nc.dram_tensor("Wg2", [D, D], f32, kind="ExternalInput")
    Wb1 = nc.dram_tensor("Wb1", [D, D], f32, kind="ExternalInput")
    Wb2 = nc.dram_tensor("Wb2", [D, D], f32, kind="ExternalInput")
    y = nc.dram_tensor("y", [SSH, 2], f32, kind="ExternalOutput")

    # DRAM scratch for partition-broadcast bounces
    k_dram = nc.dram_tensor("k_dram", [H, S], bf16)
    v_dram = nc.dram_tensor("v_dram", [H, S], bf16)

    with tile.TileContext(nc) as tc:
        with tc.tile_pool(name="big", bufs=1) as big, \
             tc.tile_pool(name="scr", bufs=4) as scrp, \
             tc.tile_pool(name="kv", bufs=1) as kvp, \
             tc.tile_pool(name="small", bufs=1) as sm, \
             tc.tile_pool(name="att", bufs=3) as att, \
             tc.tile_pool(name="psum", bufs=1, space="PSUM") as psum:

            # ---------- loads ----------
            x2big = big.tile([P, TC * D], f32)
            nc.sync.dma_start(x2big[:], x2.rearrange("(p c) d -> p (c d)", p=P))
            x1fbig = big.tile([P, TC * D], f32)
            nc.sync.dma_start(x1fbig[:], x1f.rearrange("(p c) d -> p (c d)", p=P))
            x1sbig = big.tile([P, SC * D], f32)
            nc.sync.dma_start(x1sbig[:], x1s.rearrange("(p c) d -> p (c d)", p=P))

            wg1 = sm.tile([D, D], f32)
            nc.scalar.dma_start(wg1[:], Wg1[:, :])
            wg2 = sm.tile([D, D], f32)
            nc.scalar.dma_start(wg2[:], Wg2[:, :])
            wb1 = sm.tile([D, D], f32)
            nc.scalar.dma_start(wb1[:], Wb1[:, :])
            wb2 = sm.tile([D, D], f32)
            nc.scalar.dma_start(wb2[:], Wb2[:, :])

            # All small per-partition broadcasts (qkv weight columns, Wo, bo)
            # built on PE: transpose each [64,2] weight to rows, then a small
            # ones-matmul per row broadcasts it into a slice of one PSUM tile.
            # Avoids ~15 fixed-cost DMAs through DRAM.
            from concourse.masks import make_identity
            ident = sm.tile([P, P], f32)
            make_identity(nc, ident[:])
            ones_r = sm.tile([1, P], f32)
            nc.vector.memset(ones_r[:], 1.0)
            # sel[h]: [2,128] with row h all-ones -> lhsT.T @ twr picks row h
            sel0 = sm.tile([H, P], f32)
            nc.vector.memset(sel0[:], 0.0)
            nc.vector.memset(sel0[0:1, :], 1.0)
            sel1 = sm.tile([H, P], f32)
            nc.vector.memset(sel1[:], 1.0)
            nc.vector.memset(sel1[0:1, :], 0.0)
            sel = [sel0, sel1]
            wab_ps = psum.tile([P, 6 * D + 6], f32)
            for i, Wt in enumerate((Wq, Wk, Wv)):
                t = sm.tile([D, H], f32, name=f"wtmp{i}")
                nc.scalar.dma_start(t[:], Wt[:, :])
                tp = psum.tile([H, D], f32, name=f"wtp{i}", tag="wtp")
                nc.tensor.transpose(tp[:], t[:], ident[:D, :D])
                twr = sm.tile([H, D], f32, name=f"twr{i}")
                nc.vector.tensor_copy(twr[:], tp[:])
                for h in range(H):
                    nc.tensor.matmul(wab_ps[:, (2 * i + h) * D:(2 * i + h + 1) * D],
                                     sel[h][:], twr[:], start=True, stop=True)
            wof = sm.tile([1, 4], f32)
            nc.scalar.dma_start(wof[:], Wo.rearrange("h j -> (h j)").rearrange("(o f) -> o f", o=1))
            nc.tensor.matmul(wab_ps[:, 6 * D:6 * D + 4], ones_r[:], wof[:], start=True, stop=True)
            bof = sm.tile([1, 2], f32)
            nc.scalar.dma_start(bof[:], bo[:, :])
            nc.tensor.matmul(wab_ps[:, 6 * D + 4:6 * D + 6], ones_r[:], bof[:], start=True, stop=True)
            wab = sm.tile([P, 6 * D + 6], f32)
            nc.vector.tensor_copy(wab[:], wab_ps[:])
            wq_b = [wab[:, h * D:(h + 1) * D] for h in range(H)]
            wk_b = [wab[:, (2 + h) * D:(3 + h) * D] for h in range(H)]
            wv_b = [wab[:, (4 + h) * D:(5 + h) * D] for h in range(H)]
            wo_b = {(h, j): wab[:, 6 * D + h * 2 + j:6 * D + h * 2 + j + 1]
                    for h in range(H) for j in range(2)}
            bo_b = [wab[:, 6 * D + 4 + j:6 * D + 4 + j + 1] for j in range(2)]

            ones = sm.tile([P, 1], f32)
            nc.vector.memset(ones[:], 1.0)

            # ---------- x1 mean -> h_col [64,1] ----------
            h_ps = psum.tile([D, 1], f32)
            for c in range(TC):
                nc.tensor.matmul(h_ps[:], x1fbig[:, c * D:(c + 1) * D], ones[:],
                                 start=(c == 0), stop=(c == TC - 1))
            h_col = sm.tile([D, 1], f32)
            nc.vector.tensor_scalar_mul(h_col[:], h_ps[:], 1.0 / S)

            # ---------- x2 stats: mu, E[x^2] ----------
            x2sq = big.tile([P, TC * D], f32)
            nc.gpsimd.tensor_tensor(out=x2sq[:], in0=x2big[:], in1=x2big[:], op=ALU.mult)

            mu_ps = psum.tile([1, D], f32)
            for c in range(TC):
                nc.tensor.matmul(mu_ps[:], ones[:], x2big[:, c * D:(c + 1) * D],
                                 start=(c == 0), stop=(c == TC - 1))
            msq_ps = psum.tile([1, D], f32)
            for c in range(TC):
                nc.tensor.matmul(msq_ps[:], ones[:], x2sq[:, c * D:(c + 1) * D],
                                 start=(c == 0), stop=(c == TC - 1))
            mu = sm.tile([1, D], f32)
            nc.vector.tensor_scalar_mul(mu[:], mu_ps[:], 1.0 / S)
            msq = sm.tile([1, D], f32)
            nc.vector.tensor_scalar_mul(msq[:], msq_ps[:], 1.0 / S)

            # var = msq - mu^2 ; rstd = sqrt(1/(var+eps))
            musq = sm.tile([1, D], f32)
            nc.vector.tensor_tensor(out=musq[:], in0=mu[:], in1=mu[:], op=ALU.mult)
            var = sm.tile([1, D], f32)
            nc.vector.tensor_tensor(out=var[:], in0=msq[:], in1=musq[:], op=ALU.subtract)
            nc.vector.tensor_scalar_add(var[:], var[:], EPS_BN)
            rvar = sm.tile([1, D], f32)
            nc.vector.reciprocal(rvar[:], var[:])
            rstd = sm.tile([1, D], f32)
            nc.scalar.activation(rstd[:], rvar[:], AF.Sqrt)

            # ---------- CBN MLPs: dg, db rows [1, 64] ----------
            def mlp(w1, w2, name):
                z_ps = psum.tile([D, 1], f32, name=f"z_ps_{name}", tag="z_ps")
                nc.tensor.matmul(z_ps[:], w1[:], h_col[:], start=True, stop=True)
                zr = sm.tile([D, 1], f32, name=f"zr_{name}")
                nc.scalar.activation(zr[:], z_ps[:], AF.Relu)
                d_ps = psum.tile([1, D], f32, name=f"d_ps_{name}", tag="d_ps")
                nc.tensor.matmul(d_ps[:], zr[:], w2[:], start=True, stop=True)
                return d_ps

            dg_ps = mlp(wg1, wg2, "g")
            db_ps = mlp(wb1, wb2, "b")

            # A = (1+dg)*rstd ; B = db - mu*A
            dgp1 = sm.tile([1, D], f32)
            nc.vector.tensor_scalar_add(dgp1[:], dg_ps[:], 1.0)
            A_row = sm.tile([1, D], f32)
            nc.vector.tensor_tensor(out=A_row[:], in0=dgp1[:], in1=rstd[:], op=ALU.mult)
            muA = sm.tile([1, D], f32)
            nc.vector.tensor_tensor(out=muA[:], in0=mu[:], in1=A_row[:], op=ALU.mult)
            B_row = sm.tile([1, D], f32)
            nc.vector.tensor_tensor(out=B_row[:], in0=db_ps[:], in1=muA[:], op=ALU.subtract)

            # bounce A,B through DRAM to partition-broadcast
            # broadcast A,B across partitions on PE: out = ones[1,128].T @ ab_row[1,128]
            ab_row = sm.tile([1, 2 * D], f32)
            nc.vector.tensor_copy(ab_row[:, 0:D], A_row[:])
            nc.vector.tensor_copy(ab_row[:, D:2 * D], B_row[:])
            ab_ps = psum.tile([P, 2 * D], f32)
            nc.tensor.matmul(ab_ps[:], ones_r[:], ab_row[:], start=True, stop=True)
            ab_b = sm.tile([P, 2 * D], f32)
            nc.vector.tensor_copy(ab_b[:], ab_ps[:])
            A_b = ab_b[:, 0:D]
            B_b = ab_b[:, D:2 * D]

            # ---------- v2 = x2*A + B (CBN applied) ----------
            v2 = big.tile([P, TC * D], f32)
            x2v = x2big[:].rearrange("p (c d) -> p c d", d=D)
            v2v = v2[:].rearrange("p (c d) -> p c d", d=D)
            A_rep = A_b.rearrange("p (c d) -> p c d", c=1).to_broadcast((P, TC, D))
            B_rep = B_b.rearrange("p (c d) -> p c d", c=1).to_broadcast((P, TC, D))
            nc.gpsimd.tensor_tensor(out=v2v, in0=x2v, in1=A_rep, op=ALU.mult)
            nc.gpsimd.tensor_tensor(out=v2v, in0=v2v, in1=B_rep, op=ALU.add)

            # ---------- row norms ----------
            rn2 = sm.tile([P, TC], f32)
            nc.vector.reduce_sum(rn2[:], x2sq[:].rearrange("p (c d) -> p c d", d=D),
                                 axis=mybir.AxisListType.X)
            in2 = sm.tile([P, TC], f32)
            nc.vector.reciprocal(in2[:], rn2[:])
            nc.scalar.activation(in2[:], in2[:], AF.Sqrt)

            v2sq = big.tile([P, TC * D], f32)
            nc.gpsimd.tensor_tensor(out=v2sq[:], in0=v2[:], in1=v2[:], op=ALU.mult)
            rnv = sm.tile([P, TC], f32)
            nc.vector.reduce_sum(rnv[:], v2sq[:].rearrange("p (c d) -> p c d", d=D),
                                 axis=mybir.AxisListType.X)
            inv2 = sm.tile([P, TC], f32)
            nc.vector.reciprocal(inv2[:], rnv[:])
            nc.scalar.activation(inv2[:], inv2[:], AF.Sqrt)

            x1ssq = sm.tile([P, SC * D], f32)
            nc.vector.tensor_tensor(out=x1ssq[:], in0=x1sbig[:], in1=x1sbig[:], op=ALU.mult)
            rn1 = sm.tile([P, SC], f32)
            nc.vector.reduce_sum(rn1[:], x1ssq[:].rearrange("p (c d) -> p c d", d=D),
                                 axis=mybir.AxisListType.X)
            in1 = sm.tile([P, SC], f32)
            nc.vector.reciprocal(in1[:], rn1[:])
            nc.scalar.activation(in1[:], in1[:], AF.Sqrt)

            # ---------- projections (col-form mul+reduce) ----------
            def proj(src_big, w_b, inv, n_chunks, name, out_dt=f32, meng=None):
                meng = meng or nc.vector
                """out[p, c] = inv[p,c] * sum_d src[p, c, d] * w_b[p, d]"""
                scr = scrp.tile([P, n_chunks * D], f32, tag="scr", name=f"scr_{name}")
                w_rep = w_b.rearrange("p (c d) -> p c d", c=1).to_broadcast((P, n_chunks, D))
                meng.tensor_tensor(out=scr[:].rearrange("p (c d) -> p c d", d=D),
                                   in0=src_big[:].rearrange("p (c d) -> p c d", d=D),
                                   in1=w_rep, op=ALU.mult)
                raw = sm.tile([P, n_chunks], f32, name=f"raw_{name}")
                nc.vector.reduce_sum(raw[:], scr[:].rearrange("p (c d) -> p c d", d=D),
                                     axis=mybir.AxisListType.X)
                outp = sm.tile([P, n_chunks], out_dt, name=f"proj_{name}")
                nc.vector.tensor_tensor(out=outp[:], in0=raw[:], in1=inv[:], op=ALU.mult)
                return outp

            q_hat = [proj(x1sbig, wq_b[h], in1, SC, f"q{h}") for h in range(H)]
            k_hat = [proj(x2big, wk_b[h], in2, TC, f"k{h}", out_dt=bf16, meng=nc.gpsimd) for h in range(H)]
            v_hat = [proj(v2, wv_b[h], inv2, TC, f"v{h}", out_dt=bf16) for h in range(H)]

            # ---------- broadcast k and v (bf16) across partitions ----------
            # bf16 halves the broadcast bytes; spread across SP/PE HW queues.
            k_b = []
            v_b = []
            for h in range(H):
                nc.sync.dma_start(k_dram[h:h + 1, :], k_hat[h][:])
                nc.sync.dma_start(v_dram[h:h + 1, :], v_hat[h][:])
                kb = kvp.tile([P, S], bf16, name=f"k_b{h}")
                eng = nc.sync if h == 0 else nc.gpsimd
                eng.dma_start(kb[:], k_dram[h:h + 1, :].to_broadcast((P, S)))
                k_b.append(kb)
                vb = kvp.tile([P, S], bf16, name=f"v_b{h}")
                eng = nc.sync if h == 0 else nc.gpsimd
                eng.dma_start(vb[:], v_dram[h:h + 1, :].to_broadcast((P, S)))
                v_b.append(vb)

            # ---------- attention: per (head, s-chunk) ----------
            den_all = sm.tile([P, H * SC], f32)
            num_all = sm.tile([P, H * SC], f32)
            for h in range(H):
                for sc in range(SC):
                    idx = h * SC + sc
                    e_t = att.tile([P, S], bf16, tag="e", name=f"e_{h}_{sc}")
                    nc.scalar.activation(e_t[:], k_b[h][:], AF.Exp,
                                         bias=0.0, scale=q_hat[h][:, sc:sc + 1],
                                         accum_out=den_all[:, idx:idx + 1])
                    scr = att.tile([P, S], bf16, tag="ttr_scr", name=f"ts_{h}_{sc}")
                    nc.vector.scalar_tensor_tensor(
                        out=scr[:], in0=e_t[:], scalar=1.0, in1=v_b[h][:],
                        op0=ALU.mult, op1=ALU.mult,
                        accum_out=num_all[:, idx:idx + 1])

            # ---------- epilogue: batched r, logits, sigmoid ----------
            rden_all = sm.tile([P, H * SC], f32)
            nc.vector.reciprocal(rden_all[:], den_all[:])
            r_all = sm.tile([P, H * SC], f32)
            nc.vector.tensor_tensor(out=r_all[:], in0=num_all[:], in1=rden_all[:], op=ALU.mult)
            r0 = r_all[:, 0:SC]
            r1 = r_all[:, SC:2 * SC]

            z_all = sm.tile([P, SC * 2], f32)
            zv = z_all[:].rearrange("p (c j) -> p c j", j=2)
            t2 = sm.tile([P, SC * 2], f32)
            t2v = t2[:].rearrange("p (c j) -> p c j", j=2)
            for j in range(2):
                nc.vector.tensor_scalar(out=zv[:, :, j], in0=r0,
                                        scalar1=wo_b[(0, j)], scalar2=bo_b[j],
                                        op0=ALU.mult, op1=ALU.add)
                nc.vector.tensor_scalar_mul(t2v[:, :, j], r1, wo_b[(1, j)])
            nc.vector.tensor_tensor(out=z_all[:], in0=z_all[:], in1=t2[:], op=ALU.add)

            # sigmoid(z) = 1/(1+exp(-z))  (reuses the exp table set)
            sig = sm.tile([P, SC * 2], f32)
            nc.scalar.activation(sig[:], z_all[:], AF.Exp, bias=0.0, scale=-1.0)
            nc.vector.tensor_scalar_add(sig[:], sig[:], 1.0)
            nc.vector.reciprocal(sig[:], sig[:])

            nc.sync.dma_start(y.rearrange("(p c) j -> p (c j)", p=P), sig[:])

    if split:
        _split_waits(nc, mybir)
    return nc


def _split_waits(nc, mybir, maxw=1):
    """This container's walrus build rejects instructions carrying more than
    ~2 sync-wait commands. Split excess waits onto zero-register-write nops
    inserted just before the instruction on the same engine (same-engine
    program order preserves the wait-before-execute semantics)."""
    ctr = 0
    for bb in nc.m.functions[0].blocks:
        new = []
        for inst in bb.instructions:
            si = inst.sync_info
            if si is not None and si.on_wait and len(si.on_wait) > maxw:
                waits = list(si.on_wait)
                ename = str(inst.engine).split(".")[-1]
                for w in waits[:-maxw]:
                    ctr += 1
                    new.append(mybir.InstRegisterMove(
                        name=f"WS-{ctr}",
                        ins=[mybir.ImmediateValue(kind="imm_value", dtype=mybir.dt.int32, value=0)],
                        outs=[mybir.RegisterAccess(kind="register_access", regref=f"{ename}_zero", dtype=mybir.dt.int32)],
                        engine=inst.engine,
                        sync_info=mybir.SyncInfo(on_wait=[w], on_update=[]),
                    ))
                si.on_wait = waits[-maxw:]
            new.append(inst)
        bb.instructions = new


def _get_program():
    if "nc" not in _CACHE:
        _CACHE["nc"] = _build()
    return _CACHE["nc"]


def kernel(x1, x2, Wq, Wk, Wv, Wo, bo, Wg1, Wg2, Wb1, Wb2):
    from concourse import bass_utils

    nc = _get_program()
    x1s_full = np.ascontiguousarray(x1[0])  # [4096, 64]
    x2s = np.ascontiguousarray(x2[0])

    in_maps = []
    for i in range(NCORES):
        in_maps.append({
            "x1s": np.ascontiguousarray(x1s_full[i * SSH:(i + 1) * SSH]),
            "x1f": x1s_full,
            "x2": x2s,
            "Wq": Wq, "Wk": Wk, "Wv": Wv, "Wo": Wo,
            "bo": np.ascontiguousarray(bo[None, :]),
            "Wg1": Wg1, "Wg2": Wg2, "Wb1": Wb1, "Wb2": Wb2,
        })

    # First execution of a freshly-compiled NEFF occasionally reports a
    # transient device error through the PJRT proxy; a retry succeeds.
    last_err = None
    for attempt in range(3):
        try:
            res = bass_utils.run_bass_kernel_spmd(nc, in_maps, core_ids=list(range(NCORES)))
            out = np.concatenate([res.results[i]["y"] for i in range(NCORES)], axis=0)
            return out.reshape(1, S, 2)
        except Exception as e:  # noqa: BLE001
            last_err = e
            import time
            time.sleep(5)
    raise last_err



# revision 3
# speedup vs baseline: 2.1694x; 2.1694x over previous
"""Trainium2 Bass kernel for nn_CrossFusion — polynomial-softmax rewrite.

Key idea: k_dim = 1 makes the attention scores rank-1, e[s,t] = exp(q_s*k_t),
so num(q) = sum_t v_t exp(q k_t) and den(q) = sum_t exp(q k_t) are analytic
functions of the scalar q_s.  Expanding exp in a Taylor series:
    den(q) = sum_m q^m/m! * S_m,   S_m = sum_t k_t^m
    num(q) = sum_m q^m/m! * T_m,   T_m = sum_t v_t k_t^m
With |q*k| <= ~1.2 a degree-12 truncation has ~1e-7 relative error, so the
whole [S1,S2] attention collapses to power sums over t (tiny column-form ops)
plus a per-s Horner evaluation.  No k/v broadcasts, no exp over [128,4096].

Data layout: x2 is cast to bf16 (gpsimd casting DMA) and transposed by the
XBAR DMA engine into "T-form": a [128, 2048] tile where partition p<64 holds
feature d=p of even rows t=2i and p>=64 holds feature d=p-64 of odd rows.
All projections (k, v-numerator, row norms n2, CBN'd norms nv2) then become
two stacked PE matmuls per 512-column chunk contracting the partition dim,
with the CBN affine folded into the weight columns algebraically:
    v2.Wv = x2.(A*Wv) + B.Wv ;  ||v2||^2 = x2^2.A^2 + x2.(2AB) + ||B||^2
CBN stats (mu, var of x2) come from bn_stats on the same T-form tile.
The [12, 512]-per-chunk matmul results are de-transposed back to column form
([128 partitions x 32] per quantity) with 4 PE transposes so the power-sum
chain runs as ~25 tiny [128,64] DVE/Pool ops.

x1 (query) side uses the same cast+XBAR+matmul pipeline at 1/8 scale.
Output rows are s = 256b + 2p + eo (b,eo in {0,1}, p partition).
"""
import numpy as np

S = 4096
D = 64
H = 2
NCORES = 8
SSH = S // NCORES   # 512 query rows per core
M = 12              # Taylor degree
NC_CH = 4           # x2 chunks (cast/transpose/square/matmul pipeline)
EPS_BN = 1e-5

_CACHE = {}


def _build(split=True):
    import concourse.bass as bass
    import concourse.tile as tile
    import concourse.mybir as mybir
    from concourse.masks import make_identity

    f32 = mybir.dt.float32
    bf16 = mybir.dt.bfloat16
    AF = mybir.ActivationFunctionType
    ALU = mybir.AluOpType
    P = 128
    NI = S // 2       # 2048 i-indices (t = 2i + eo)
    CW = NI // NC_CH  # 512 columns per chunk

    nc = bass.Bass("TRN2", target_bir_lowering=False, debug=False)

    x1s = nc.dram_tensor("x1s", [SSH, D], f32, kind="ExternalInput")
    x1f = nc.dram_tensor("x1f", [S, D], f32, kind="ExternalInput")
    x2 = nc.dram_tensor("x2", [S, D], f32, kind="ExternalInput")
    Wq = nc.dram_tensor("Wq", [D, H], f32, kind="ExternalInput")
    Wk = nc.dram_tensor("Wk", [D, H], f32, kind="ExternalInput")
    Wv = nc.dram_tensor("Wv", [D, H], f32, kind="ExternalInput")
    Wo = nc.dram_tensor("Wo", [H, 2], f32, kind="ExternalInput")
    bo = nc.dram_tensor("bo", [1, 2], f32, kind="ExternalInput")
    Wg1 = nc.dram_tensor("Wg1", [D, D], f32, kind="ExternalInput")
    Wg2 = nc.dram_tensor("Wg2", [D, D], f32, kind="ExternalInput")
    Wb1 = nc.dram_tensor("Wb1", [D, D], f32, kind="ExternalInput")
    Wb2 = nc.dram_tensor("Wb2", [D, D], f32, kind="ExternalInput")
    y = nc.dram_tensor("y", [SSH, 2], f32, kind="ExternalOutput")

    # bf16 DRAM scratch for the casting DMAs feeding the XBAR transpose
    x2bf = nc.dram_tensor("x2bf", [S, D], bf16)
    x1sbf = nc.dram_tensor("x1sbf", [SSH, D], bf16)

    with tile.TileContext(nc) as tc:
        with tc.tile_pool(name="sb", bufs=1) as sb, \
             tc.tile_pool(name="psum", bufs=1, space="PSUM") as psum:

            # ---------------- PSUM allocation (banks are 2KB granular) ------
            # PP: dense [12, 2048]: row q, col i (global); each chunk matmul
            # writes one 512-col range = exactly one bank.
            PP = psum.tile([12, NI], f32, name="PP")
            TPbig = psum.tile([P, 256], bf16, name="TPbig")  # de-transpose outs
            TP = TPbig[:, 0:192]
            TQ = TPbig[:, 192:204]
            PQm = psum.tile([4, 2 * (SSH // 2)], f32, name="PQm")
            PQ = PQm[0:4, 0:SSH // 2]
            PQ2 = PQm[0:2, SSH // 2:2 * (SSH // 2)]
            SM = psum.tile([P, 160], f32, name="SM")         # shared bank for small psums
            h_ps = SM[0:64, 0:1]
            zg_ps = SM[0:64, 1:2]
            zb_ps = SM[0:64, 2:3]
            dg_ps = SM[:, 3:4]
            db_ps = SM[:, 4:5]
            stats_ps = SM[:, 5:7]
            consts_ps = SM[0:1, 8:11]
            coef_ps = SM[0:1, 11:11 + 4 * (M + 1)]
            cb9_ps = SM[:, 63:72]
            cb52_ps = SM[:, 72:72 + 4 * (M + 1)]

            # ---------------- SBUF tiles -----------------------------------
            xstack = sb.tile([P, NI], bf16)       # x2 T-form (even|odd partition split)
            xsq = sb.tile([P, NI], bf16)          # xstack^2
            x1sT = sb.tile([P, SSH // 2], bf16)
            x1sq = sb.tile([P, SSH // 2], bf16)
            x1fbig = sb.tile([P, (S // P) * D], f32)
            wg1_sb = sb.tile([D, D], f32)
            wg2_sb = sb.tile([D, D], f32)
            wb1_sb = sb.tile([D, D], f32)
            wb2_sb = sb.tile([D, D], f32)
            wv2 = sb.tile([P, H], f32)            # Wv duplicated on both halves
            worow = sb.tile([1, 4], f32)
            borow = sb.tile([1, 2], f32)
            lhsT1 = sb.tile([P, 12], bf16)        # x-part weights
            lhsT2 = sb.tile([P, 12], bf16)        # x^2-part weights
            lhsTq = sb.tile([P, 6], bf16)
            ident = sb.tile([P, P], bf16)
            Mh = sb.tile([P, P], f32)             # even/odd stat combiner
            ones_col = sb.tile([P, 1], f32)
            ones_bf = sb.tile([P, 1], bf16)
            ones_row = sb.tile([1, P], f32)
            junk = sb.tile([1, 1], f32)
            eps_col = sb.tile([P, 1], f32)
            bnraw = sb.tile([P, 6 * NC_CH], f32)
            bnagg = sb.tile([P, 2], f32)
            musq = sb.tile([P, 1], f32)
            mu128s = sb.tile([P, 1], f32)
            var128 = sb.tile([P, 1], f32)
            lnv = sb.tile([P, 1], f32)
            rs128 = sb.tile([P, 1], f32)
            A128 = sb.tile([P, 1], f32)
            muA = sb.tile([P, 1], f32)
            B128 = sb.tile([P, 1], f32)
            zg_sb = sb.tile([D, 1], f32)
            zb_sb = sb.tile([D, 1], f32)
            h_col = sb.tile([D, 1], f32)
            crow = sb.tile([1, 9], f32)
            constsb = sb.tile([P, 9], f32)
            PPs = sb.tile([12, NI], bf16)
            C = sb.tile([P, 192], f32)
            PQs = sb.tile([4, SSH // 2], bf16)
            PQs2 = sb.tile([2, SSH // 2], bf16)
            C1 = sb.tile([P, 12], f32)
            rsn2 = sb.tile([P, 32], f32)
            lt2 = sb.tile([P, 32], f32)
            rsnv = sb.tile([P, 32], f32)
            lt3 = sb.tile([P, 32], f32)
            rsq1 = sb.tile([P, 4], f32)
            lt1 = sb.tile([P, 4], f32)
            K_all = sb.tile([P, (M + 1) * 2 * 64], bf16)
            R = sb.tile([P, 4 * (M + 1)], f32)
            coefrow = sb.tile([1, 4 * (M + 1)], f32)
            cbrow = sb.tile([P, 4 * (M + 1)], f32)
            qhat = sb.tile([P, 8], f32)
            acc = sb.tile([P, 16], f32)
            rden = sb.tile([P, 8], f32)
            rr = sb.tile([P, 8], f32)
            zt = sb.tile([P, 8], f32)
            t2 = sb.tile([P, 8], f32)
            ez = sb.tile([P, 8], f32)
            sig = sb.tile([P, 8], f32)

            # ============ t=0: DMAs and static prep =========================
            # gpsimd (SWDGE, casts): x1s first (small), then x2 chunks, weights
            nc.gpsimd.dma_start(x1sbf[:, :], x1s[:, :])
            for c in range(NC_CH):
                r0, r1 = c * (S // NC_CH), (c + 1) * (S // NC_CH)
                nc.gpsimd.dma_start(x2bf[r0:r1, :], x2[r0:r1, :])
            nc.gpsimd.dma_start(lhsTq[0:64, 0:3:2], Wq[:, :])
            nc.gpsimd.dma_start(lhsTq[64:128, 1:4:2], Wq[:, :])
            nc.gpsimd.dma_start(lhsT1[0:64, 0:3:2], Wk[:, :])
            nc.gpsimd.dma_start(lhsT1[64:128, 1:4:2], Wk[:, :])

            # SP (sync) queue: XBAR transposes (x1s, then x2 chunks)
            vq = x1sbf.rearrange("(i two) d -> i (two d)", two=2)
            nc.sync.dma_start_transpose(x1sT[:], vq[:, :])
            v2d = x2bf.rearrange("(i two) d -> i (two d)", two=2)
            for c in range(NC_CH):
                nc.sync.dma_start_transpose(xstack[:, c * CW:(c + 1) * CW],
                                            v2d[c * CW:(c + 1) * CW, :])

            # scalar (ACT) queue: table preload + f32 loads
            nc.vector.memset(junk[:], 0.0)
            nc.scalar.activation(junk[:], junk[:], AF.Exp)  # preload ln/exp table
            nc.scalar.dma_start(x1fbig[:], x1f.rearrange("(p c) d -> p (c d)", p=P))
            nc.scalar.dma_start(wg1_sb[:], Wg1[:, :])
            nc.scalar.dma_start(wb1_sb[:], Wb1[:, :])
            nc.scalar.dma_start(wg2_sb[:], Wg2[:, :])
            nc.scalar.dma_start(wb2_sb[:], Wb2[:, :])

            # gpsimd queue: small weight loads (Pool SEQ dispatch is ~25ns)
            nc.gpsimd.dma_start(wv2[0:64, :], Wv[:, :])
            nc.gpsimd.dma_start(wv2[64:128, :], Wv[:, :])
            nc.gpsimd.dma_start(worow[:], Wo.rearrange("h j -> (h j)").rearrange("(o f) -> o f", o=1))
            nc.gpsimd.dma_start(borow[:], bo[:, :])

            # static SBUF prep (DVE + Pool)
            nc.vector.memset(ones_col[:], 1.0)
            nc.vector.memset(ones_bf[:], 1.0)
            nc.vector.memset(ones_row[:], 1.0)
            nc.vector.memset(eps_col[:], EPS_BN)
            make_identity(nc, ident[:])           # gpsimd x2
            nc.gpsimd.memset(Mh[:], 0.0)
            nc.gpsimd.tensor_scalar_mul(Mh[0:64, 0:64], ident[0:64, 0:64], 0.5)
            nc.gpsimd.tensor_scalar_mul(Mh[64:128, 0:64], ident[64:128, 64:128], 0.5)
            nc.gpsimd.tensor_copy(Mh[:, 64:128], Mh[:, 0:64])
            nc.vector.memset(lhsTq[:], 0.0)       # before the Wq casts (tile dep)
            nc.vector.memset(lhsT1[:], 0.0)
            nc.vector.memset(lhsT2[:], 0.0)
            nc.vector.memset(lhsTq[0:64, 4:5], 1.0)
            nc.vector.memset(lhsTq[64:128, 5:6], 1.0)
            nc.vector.memset(lhsT2[0:64, 8:9], 1.0)
            nc.vector.memset(lhsT2[64:128, 9:10], 1.0)
            def kslice(m):
                return K_all[:, (2 * m) * 64:(2 * m + 1) * 64]

            def uslice(m):
                return K_all[:, (2 * m + 1) * 64:(2 * m + 2) * 64]

            nc.vector.memset(kslice(0), 1.0)  # k~_0 = 1

            # ============ x1f mean -> h, CBN MLPs (PE) ======================
            TCf = S // P
            for c in range(TCf):
                nc.tensor.matmul(h_ps[:], x1fbig[:, c * D:(c + 1) * D], ones_col[:],
                                 start=(c == 0), stop=(c == TCf - 1))
            nc.vector.tensor_scalar_mul(h_col[:], h_ps[:], 1.0 / S)
            nc.tensor.matmul(zg_ps[:], wg1_sb[:], h_col[:], start=True, stop=True)
            nc.tensor.matmul(zb_ps[:], wb1_sb[:], h_col[:], start=True, stop=True)
            nc.scalar.activation(zg_sb[:], zg_ps[:], AF.Relu)
            nc.scalar.activation(zb_sb[:], zb_ps[:], AF.Relu)
            nc.tensor.matmul(dg_ps[0:64, :], wg2_sb[:], zg_sb[:], start=True, stop=True)
            nc.tensor.matmul(dg_ps[64:128, :], wg2_sb[:], zg_sb[:], start=True, stop=True)
            nc.tensor.matmul(db_ps[0:64, :], wb2_sb[:], zb_sb[:], start=True, stop=True)
            nc.tensor.matmul(db_ps[64:128, :], wb2_sb[:], zb_sb[:], start=True, stop=True)

            # ============ x1 side: square, proj, de-transpose ===============
            nc.scalar.activation(x1sq[:], x1sT[:], AF.Square)
            nc.tensor.matmul(PQ[:], lhsTq[:, 0:4], x1sT[:], start=True, stop=True)
            nc.tensor.matmul(PQ2[:], lhsTq[:, 4:6], x1sq[:], start=True, stop=True)
            nc.scalar.copy(PQs[:], PQ[0:4, :])
            nc.scalar.copy(PQs2[:], PQ2[:])
            for b in range(2):
                nc.tensor.transpose(TQ[:, 6 * b:6 * b + 4], PQs[:, 128 * b:128 * (b + 1)],
                                    ident[0:4, 0:4])
                nc.tensor.transpose(TQ[:, 6 * b + 4:6 * b + 6], PQs2[:, 128 * b:128 * (b + 1)],
                                    ident[0:2, 0:2])
            nc.scalar.copy(C1[:], TQ[:])
            C1v = C1[:].rearrange("p (b q) -> p b q", b=2)
            nc.scalar.activation(lt1[:], C1v[:, :, 4:6], AF.Ln)
            nc.scalar.activation(rsq1[:], lt1[:], AF.Exp, scale=-0.5)
            rsq1v = rsq1[:].rearrange("p (b e) -> p b e", b=2)
            for hh in range(H):
                nc.vector.tensor_tensor(
                    out=qhat[:, 4 * hh:4 * (hh + 1)].rearrange("p (b e) -> p b e", b=2),
                    in0=C1v[:, :, 2 * hh:2 * hh + 2], in1=rsq1v, op=ALU.mult)

            # ============ x2 stats (bn_stats on T-form) =====================
            for c in range(NC_CH):
                nc.vector.bn_stats(bnraw[:, 6 * c:6 * (c + 1)], xstack[:, c * CW:(c + 1) * CW])
            nc.vector.bn_aggr(bnagg[:], bnraw[:].rearrange("p (g t) -> p g t", t=3))
            # msq_half = var + mean^2 per partition (even/odd separately)
            nc.vector.tensor_tensor(out=musq[:], in0=bnagg[:, 0:1], in1=bnagg[:, 0:1], op=ALU.mult)
            nc.vector.tensor_tensor(out=var128[:], in0=bnagg[:, 1:2], in1=musq[:], op=ALU.add)
            # combine even/odd halves; duplicate onto all 128 partitions via PE
            nc.tensor.matmul(stats_ps[:, 0:1], Mh[:], bnagg[:, 0:1], start=True, stop=True)
            nc.tensor.matmul(stats_ps[:, 1:2], Mh[:], var128[:], start=True, stop=True)
            nc.vector.tensor_tensor(out=musq[:], in0=stats_ps[:, 0:1], in1=stats_ps[:, 0:1], op=ALU.mult)
            nc.vector.tensor_tensor(out=var128[:], in0=stats_ps[:, 1:2], in1=musq[:], op=ALU.subtract)
            nc.scalar.activation(lnv[:], var128[:], AF.Ln, bias=eps_col[:])
            nc.scalar.activation(rs128[:], lnv[:], AF.Exp, scale=-0.5)
            nc.vector.scalar_tensor_tensor(out=A128[:], in0=dg_ps[:], scalar=1.0,
                                           in1=rs128[:], op0=ALU.add, op1=ALU.mult)
            nc.vector.tensor_tensor(out=muA[:], in0=stats_ps[:, 0:1], in1=A128[:], op=ALU.mult)
            nc.vector.tensor_tensor(out=B128[:], in0=db_ps[:], in1=muA[:], op=ALU.subtract)

            # lhsT columns that depend on A, B
            for hh in range(H):
                nc.vector.tensor_tensor(out=lhsT1[0:64, 4 + 2 * hh:5 + 2 * hh],
                                        in0=A128[0:64, :], in1=wv2[0:64, hh:hh + 1], op=ALU.mult)
                nc.vector.tensor_tensor(out=lhsT1[64:128, 5 + 2 * hh:6 + 2 * hh],
                                        in0=A128[64:128, :], in1=wv2[64:128, hh:hh + 1], op=ALU.mult)
            nc.vector.scalar_tensor_tensor(out=lhsT1[0:64, 10:11], in0=A128[0:64, :], scalar=2.0,
                                           in1=B128[0:64, :], op0=ALU.mult, op1=ALU.mult)
            nc.vector.scalar_tensor_tensor(out=lhsT1[64:128, 11:12], in0=A128[64:128, :], scalar=2.0,
                                           in1=B128[64:128, :], op0=ALU.mult, op1=ALU.mult)
            nc.vector.tensor_tensor(out=lhsT2[0:64, 10:11], in0=A128[0:64, :],
                                    in1=A128[0:64, :], op=ALU.mult)
            nc.vector.tensor_tensor(out=lhsT2[64:128, 11:12], in0=A128[64:128, :],
                                    in1=A128[64:128, :], op=ALU.mult)

            # consts: cBv_h = B.Wv_h, cB2 = B.B ; broadcast with Wo, bo
            nc.tensor.matmul(consts_ps[:, 0:2], B128[0:64, :], wv2[0:64, :], start=True, stop=True)
            nc.tensor.matmul(consts_ps[:, 2:3], B128[0:64, :], B128[0:64, :], start=True, stop=True)
            nc.vector.tensor_copy(crow[:, 0:3], consts_ps[:])
            nc.vector.tensor_copy(crow[:, 3:7], worow[:])
            nc.vector.tensor_copy(crow[:, 7:9], borow[:])
            nc.tensor.matmul(cb9_ps[:], ones_row[:], crow[:], start=True, stop=True)
            nc.scalar.copy(constsb[:], cb9_ps[:])

            # ============ x2 squares + stacked projections ==================
            for c in range(NC_CH):
                nc.scalar.activation(xsq[:, c * CW:(c + 1) * CW],
                                     xstack[:, c * CW:(c + 1) * CW], AF.Square)
            for c in range(NC_CH):
                cs = slice(c * CW, (c + 1) * CW)
                nc.tensor.matmul(PP[:, cs], lhsT1[:], xstack[:, cs],
                                 start=True, stop=False)
                nc.tensor.matmul(PP[:, cs], lhsT2[:], xsq[:, cs],
                                 start=False, stop=True)

            # ============ de-transpose to column form =======================
            nc.scalar.copy(PPs[:], PP[:])
            for g in range(16):
                nc.tensor.transpose(TP[:, 12 * g:12 * (g + 1)],
                                    PPs[:, 128 * g:128 * (g + 1)], ident[0:12, 0:12])
            nc.scalar.copy(C[:], TP[:])
            Cv = C[:].rearrange("p (g q) -> p g q", g=16)

            # ============ rsqrt norms, k^, v^ ===============================
            lt2v = lt2[:].rearrange("p (g e) -> p g e", g=16)
            lt3v = lt3[:].rearrange("p (g e) -> p g e", g=16)
            nc.scalar.activation(lt2v, Cv[:, :, 8:10], AF.Ln)
            nc.scalar.activation(rsn2[:], lt2[:], AF.Exp, scale=-0.5)
            nc.scalar.activation(lt3v, Cv[:, :, 10:12], AF.Ln, bias=constsb[:, 2:3])
            nc.scalar.activation(rsnv[:], lt3[:], AF.Exp, scale=-0.5)

            khat = kslice(1)
            vhat = uslice(0)
            nc.vector.tensor_tensor(
                out=khat.rearrange("p (h g e) -> p g h e", h=2, g=16),
                in0=Cv[:, :, 0:4].rearrange("p g (h e) -> p g h e", h=2),
                in1=rsn2[:].rearrange("p (g o e) -> p g o e", g=16, o=1)
                    .to_broadcast((P, 16, 2, 2)),
                op=ALU.mult)
            rsnvv = rsnv[:].rearrange("p (g e) -> p g e", g=16)
            for hh in range(H):
                nc.vector.scalar_tensor_tensor(
                    out=vhat[:, 32 * hh:32 * (hh + 1)].rearrange("p (g e) -> p g e", g=16),
                    in0=Cv[:, :, 4 + 2 * hh:6 + 2 * hh], scalar=constsb[:, hh:hh + 1],
                    in1=rsnvv, op0=ALU.add, op1=ALU.mult)

            # ============ power chain + reduction ===========================
            for m in range(2, M + 1):
                nc.vector.scalar_tensor_tensor(out=kslice(m), in0=kslice(m - 1),
                                               scalar=1.0 / m, in1=khat,
                                               op0=ALU.mult, op1=ALU.mult)
            for m in range(1, M + 1):
                nc.gpsimd.tensor_tensor(out=uslice(m), in0=kslice(m),
                                        in1=vhat, op=ALU.mult)
            nc.vector.reduce_sum(R[:], K_all[:].rearrange("p (g x) -> p g x", x=32),
                                 axis=mybir.AxisListType.X)
            nc.tensor.matmul(coef_ps[:], ones_col[:], R[:], start=True, stop=True)
            nc.vector.tensor_copy(coefrow[:], coef_ps[:])
            nc.tensor.matmul(cb52_ps[:], ones_row[:], coefrow[:], start=True, stop=True)
            nc.scalar.copy(cbrow[:], cb52_ps[:])

            # ============ Horner evaluation =================================
            accv = acc[:].rearrange("p (u h c) -> p u h c", u=2, h=2)
            qx = qhat[:].rearrange("p (o h c) -> p o h c", o=1, h=2).to_broadcast((P, 2, 2, 4))

            def cbm(m):
                return (cbrow[:, 4 * m:4 * (m + 1)]
                        .rearrange("p (u h o) -> p u h o", u=2, h=2)
                        .to_broadcast((P, 2, 2, 4)))

            nc.vector.tensor_copy(accv, cbm(M))
            for m in range(M - 1, -1, -1):
                nc.vector.tensor_tensor(out=accv, in0=accv, in1=qx, op=ALU.mult)
                nc.vector.tensor_tensor(out=accv, in0=accv, in1=cbm(m), op=ALU.add)

            # ============ epilogue: r, logits, sigmoid, store ===============
            nc.vector.reciprocal(rden[:], acc[:, 0:8])
            nc.vector.tensor_tensor(out=rr[:], in0=acc[:, 8:16], in1=rden[:], op=ALU.mult)
            r0 = rr[:, 0:4].rearrange("p (b e o) -> p b e o", b=2, o=1)
            r1 = rr[:, 4:8].rearrange("p (b e o) -> p b e o", b=2, o=1)
            Zv = zt[:].rearrange("p (b e j) -> p b e j", b=2, e=2)
            T2v = t2[:].rearrange("p (b e j) -> p b e j", b=2, e=2)
            for j in range(2):
                nc.vector.tensor_scalar(out=Zv[:, :, :, j:j + 1],
                                        in0=r0,
                                        scalar1=constsb[:, 3 + j:4 + j],
                                        scalar2=constsb[:, 7 + j:8 + j],
                                        op0=ALU.mult, op1=ALU.add)
                nc.vector.tensor_scalar_mul(T2v[:, :, :, j:j + 1],
                                            r1,
                                            constsb[:, 5 + j:6 + j])
            nc.vector.tensor_tensor(out=zt[:], in0=zt[:], in1=t2[:], op=ALU.add)
            nc.scalar.activation(ez[:], zt[:], AF.Exp, scale=-1.0)
            nc.vector.tensor_scalar_add(ez[:], ez[:], 1.0)
            nc.vector.reciprocal(sig[:], ez[:])
            nc.sync.dma_start(y.rearrange("(b p e) j -> p b e j", b=2, p=P),
                              sig[:].rearrange("p (b e j) -> p b e j", b=2, e=2))

    if split:
        _split_waits(nc, mybir)
    return nc


def _split_waits(nc, mybir, maxw=1):
    """This container's walrus build rejects instructions carrying more than
    ~2 sync-wait commands. Split excess waits onto zero-register-write nops
    inserted just before the instruction on the same engine (same-engine
    program order preserves the wait-before-execute semantics)."""
    ctr = 0
    for bb in nc.m.functions[0].blocks:
        new = []
        for inst in bb.instructions:
            si = inst.sync_info
            if si is not None and si.on_wait and len(si.on_wait) > maxw:
                waits = list(si.on_wait)
                ename = str(inst.engine).split(".")[-1]
                for w in waits[:-maxw]:
                    ctr += 1
                    new.append(mybir.InstRegisterMove(
                        name=f"WS-{ctr}",
                        ins=[mybir.ImmediateValue(kind="imm_value", dtype=mybir.dt.int32, value=0)],
                        outs=[mybir.RegisterAccess(kind="register_access", regref=f"{ename}_zero", dtype=mybir.dt.int32)],
                        engine=inst.engine,
                        sync_info=mybir.SyncInfo(on_wait=[w], on_update=[]),
                    ))
                si.on_wait = waits[-maxw:]
            new.append(inst)
        bb.instructions = new


def _get_program():
    if "nc" not in _CACHE:
        _CACHE["nc"] = _build()
    return _CACHE["nc"]


def kernel(x1, x2, Wq, Wk, Wv, Wo, bo, Wg1, Wg2, Wb1, Wb2):
    from concourse import bass_utils

    nc = _get_program()
    x1s_full = np.ascontiguousarray(x1[0])  # [4096, 64]
    x2s = np.ascontiguousarray(x2[0])

    in_maps = []
    for i in range(NCORES):
        in_maps.append({
            "x1s": np.ascontiguousarray(x1s_full[i * SSH:(i + 1) * SSH]),
            "x1f": x1s_full,
            "x2": x2s,
            "Wq": Wq, "Wk": Wk, "Wv": Wv, "Wo": Wo,
            "bo": np.ascontiguousarray(bo[None, :]),
            "Wg1": Wg1, "Wg2": Wg2, "Wb1": Wb1, "Wb2": Wb2,
        })

    # First execution of a freshly-compiled NEFF occasionally reports a
    # transient device error through the PJRT proxy; a retry succeeds.
    last_err = None
    for attempt in range(3):
        try:
            res = bass_utils.run_bass_kernel_spmd(nc, in_maps, core_ids=list(range(NCORES)))
            out = np.concatenate([_unscramble(res.results[i]["y"]) for i in range(NCORES)], axis=0)
            return out.reshape(1, S, 2)
        except Exception as e:  # noqa: BLE001
            last_err = e
            import time
            time.sleep(5)
    raise last_err


def _unscramble(y_core):
    # kernel writes y rows in natural s order already: s = 256b + 2p + e
    return y_core
